# revision 47
# baseline (speedup 1.0000x reference)
"""Attention kernel (out, probs) for Trainium2, 8-core SPMD.

Problem: B=2, H=16, S=2048, D=64 fp32 attention with a 0/1 key mask,
returning BOTH the attention output [B,H,S,D] and the full softmax
probabilities [B,H,S,S].

Sharding: the 32 (b,h) pairs are split 4-per-core (head parallel, no
cross-core communication).

Per-core algorithm, per head (S=2048 split into 4 q-chunks of 512):
  1. S pass: scores [128q, 2048k] via matmul(lhsT=QTe, rhs=KTe) where
     QTe=[Q^T; ones] and KTe=[K^T; maskterm] carry an extra contraction
     row adding maskterm[k] = (mask[k]-1)*8e9 to every score (masked
     keys underflow to exactly 0 after exp).  ScalarE exp(0.125*x) ->
     unnormalized expS tiles (float32).
  2. Transposed copies expST[k,q] needed for the P.V contraction come
     from two sources (balancing ScalarE vs PE/VectorE):
       - K_ACT tiles: recompute scores^T via matmul(lhsT=KTe, rhs=QTe)
         + a second ScalarE exp (writes float32r directly),
       - K_TR tiles: PE-transpose 128x128 blocks of expS into PSUM,
         VectorE-copy to SBUF as float32r.
  3. PV: outT[65, 512q] = sum_k [V | 1]^T @ expST -- the ones column
     yields the softmax denominators for free.
  4. PE-transpose outT chunks -> [128q, 65]; VectorE reciprocal of the
     denominator column; out rows scaled by recip -> DMA.
  5. probs = expS * recip[q] in-place on VectorE -> contiguous 1 MiB
     DMA per 128-row q-tile.

All matmuls run in float32r (TF32-like, 1 cycle/row); resulting rel
error vs the fp32 reference is ~2e-4, far inside the 2e-2 gate.
"""

import sys

try:
    import concourse.bass  # noqa: F401
except ImportError:
    sys.path.insert(0, "/opt/trn_rl_repo")

from contextlib import ExitStack

import numpy as np

import concourse.mybir as mybir
import concourse.tile as tile
from concourse import bacc
from concourse.bass_utils import run_bass_kernel_spmd
from concourse.masks import make_identity

B, H, S, D = 2, 16, 2048, 64
N_CORES = 8
HPC = (B * H) // N_CORES  # heads per core = 4
P = 128
QC = 512                  # q-chunk size
NQC = S // QC             # 4 q-chunks per head
NKT = S // P              # 16 k-tiles of 128
E = D + 1                 # 65: V columns + ones column
F32 = mybir.dt.float32
F32R = mybir.dt.float32r
EXP = mybir.ActivationFunctionType.Exp
SCALE = 0.125             # 1/sqrt(D)
MASKVAL = -8.0e9          # * SCALE = -1e9, matching the reference NEG
K_TR = 7                  # kt tiles per q-chunk via transpose path (rest via 2nd exp)

_CACHE = {}


def _build_bass(k_tr=K_TR):
    k_act = NKT - k_tr    # kt tiles via matmul+exp path
    nc = bacc.Bacc(None)
    qte_d = nc.dram_tensor("qte", [HPC, E, S], F32R, kind="ExternalInput")
    kte_d = nc.dram_tensor("kte", [HPC, E, S], F32R, kind="ExternalInput")
    vex_d = nc.dram_tensor("vex", [HPC, P, NKT * E], F32R, kind="ExternalInput")
    probs_d = nc.dram_tensor("probs", [HPC, S, S], F32, kind="ExternalOutput")
    out_d = nc.dram_tensor("out", [HPC, S, D], F32, kind="ExternalOutput")

    with tile.TileContext(nc) as tc, ExitStack() as ctx:
        const_pool = ctx.enter_context(tc.tile_pool(name="const", bufs=1))
        head_pool = ctx.enter_context(tc.tile_pool(name="head", bufs=3))
        # PSUM budget (8 banks):
        #   scores [128,1024] x2 bufs                      = 4 banks
        #   trdst  [128, 512] x2 bufs                      = 2 banks
        #   outt   [128, 512] x2 (shared with otr fixups)  = 2 banks
        scores_ps = ctx.enter_context(tc.tile_pool(name="scores", bufs=2, space="PSUM"))
        trdst_ps = ctx.enter_context(tc.tile_pool(name="trdst", bufs=2, space="PSUM"))
        outt_ps = ctx.enter_context(tc.tile_pool(name="outt", bufs=2, space="PSUM"))
        exps_pool = ctx.enter_context(tc.tile_pool(name="exps", bufs=10))
        expst_pool = ctx.enter_context(tc.tile_pool(name="expst", bufs=2 * NKT + 2))
        small_pool = ctx.enter_context(tc.tile_pool(name="small", bufs=2))

        ident = const_pool.tile([P, P], F32)
        make_identity(nc, ident)
        # warm the ACT exp table during the input-load window so the first
        # real exp doesn't pay the ACT_TABLE_LOAD inline
        warm = const_pool.tile([1, 1], F32)
        nc.vector.memset(warm[:], 0.0)
        nc.scalar.activation(warm[:], warm[:], EXP)

        def phase_A(qte, kte, qs, w):
            # S pass: scores [128q, 1024k] tiles + exp (unnormalized)
            exps = []  # w//P tiles of [128q, 2048k] fp32, one per q-tile
            for t in range(w // P):
                qt0 = qs + t * P
                es = exps_pool.tile([P, S], F32, tag="exps")
                for half in range(2):
                    sps = scores_ps.tile([P, 2 * QC], F32, tag="sc")
                    for j in range(2):
                        ks = (half * 2 + j) * QC
                        nc.tensor.matmul(
                            sps[:, j * QC:(j + 1) * QC],
                            qte[:, qt0:qt0 + P],
                            kte[:, ks:ks + QC],
                            start=True,
                            stop=True,
                        )
                    nc.scalar.activation(
                        es[:, half * 2 * QC:(half + 1) * 2 * QC],
                        sps[:],
                        EXP,
                        scale=SCALE,
                    )
                exps.append(es)
            return exps

        def phase_B1(qte, kte, qs, w):
            # expST tiles [128k, w], matmul + 2nd exp path for kt < k_act
            expst = [None] * NKT
            for kp in range((k_act + 1) // 2):
                st = scores_ps.tile([P, 2 * QC], F32, tag="sc")
                n_in_pair = min(2, k_act - 2 * kp)
                for j in range(n_in_pair):
                    kt = 2 * kp + j
                    nc.tensor.matmul(
                        st[:, j * w:(j + 1) * w],
                        kte[:, kt * P:(kt + 1) * P],
                        qte[:, qs:qs + w],
                        start=True,
                        stop=True,
                    )
                est = expst_pool.tile(
                    [P, 2 * QC], F32R, tag="est", bufs=(k_act + 1) // 2 + 3
                )
                nc.scalar.activation(
                    est[:, :n_in_pair * w], st[:, :n_in_pair * w], EXP, scale=SCALE
                )
                for j in range(n_in_pair):
                    expst[2 * kp + j] = est[:, j * w:(j + 1) * w]
            return expst

        def phase_B2(exps, expst, w):
            # PE-transpose path for the kt tiles B1 didn't fill -- emitted one
            # pipeline step AFTER phase_A so the transposes never wait on
            # same-step ACT
            for kt in range(NKT):
                if expst[kt] is not None:
                    continue
                td = trdst_ps.tile([P, QC], F32, tag="trdst")
                for t in range(w // P):
                    nc.tensor.transpose(
                        td[:, t * P:(t + 1) * P],
                        exps[t][:, kt * P:(kt + 1) * P],
                        ident[:],
                    )
                etr = expst_pool.tile([P, QC], F32R, tag="etr", bufs=k_tr + 2)
                nc.vector.tensor_copy(etr[:, :w], td[:, :w])
                expst[kt] = etr[:, :w]

        def phase_C1(vex, h, qs, exps, expst, w):
            # PV: outT[65, w] = sum_k [V|1]^T expST; the same PSUM bank is
            # reused afterwards for the [128, 65] out-transposes.
            fix = outt_ps.tile([P, QC], F32, tag="outt")
            outt = fix[:E, :w]
            for kt in range(NKT):
                nc.tensor.matmul(
                    outt,
                    vex[:, kt * E:(kt + 1) * E],
                    expst[kt],
                    start=(kt == 0),
                    stop=(kt == NKT - 1),
                )
            outt_sb = small_pool.tile([E, QC], F32, tag="outt_sb")
            nc.vector.tensor_copy(outt_sb[:, :w], outt)

            # transpose fixup + denominators + out (batched small ops + DMA)
            nt = w // P
            recip = small_pool.tile([P, NQC], F32, tag="recip")
            outsb = small_pool.tile([P, NQC * D], F32, tag="outsb")
            for t in range(nt):
                nc.tensor.transpose(
                    fix[:, t * E:(t + 1) * E],
                    outt_sb[:, t * P:(t + 1) * P],
                    ident[:E, :E],
                )
            fix3 = fix[:, :nt * E].rearrange("p (t e) -> p t e", e=E)
            nc.vector.reciprocal(recip[:, :nt], fix3[:, :, D])
            nc.vector.tensor_mul(
                outsb[:, :nt * D].rearrange("p (t d) -> p t d", d=D),
                fix3[:, :, :D],
                recip[:, :nt, None].to_broadcast((P, nt, D)),
            )
            nc.sync.dma_start(
                out_d[h, qs:qs + w, :].rearrange("(t p) d -> p t d", p=P),
                outsb[:, :nt * D].rearrange("p (t d) -> p t d", d=D),
            )
            return recip

        def phase_C2t(h, qs, exps, recip, t):
            # probs: normalize expS in place, DMA out (one q-tile)
            qt0 = qs + t * P
            nc.vector.tensor_scalar_mul(
                exps[t][:], exps[t][:], recip[:, t:t + 1]
            )
            nc.sync.dma_start(probs_d[h, qt0:qt0 + P, :], exps[t][:])

        # Software-pipelined emission over (head, q-offset, width) steps: the
        # final q-chunk is split into two 256-wide steps to shorten the
        # pipeline drain (PV/normalize/DMA of the very last unit).
        tiles = {}

        def load_head(h):
            qte = head_pool.tile([E, S], F32R, tag="qte", name=f"qte{h}")
            kte = head_pool.tile([E, S], F32R, tag="kte", name=f"kte{h}")
            for lo, hi in ((0, S // 2), (S // 2, S)):
                nc.sync.dma_start(kte[:, lo:hi], kte_d[h, :, lo:hi])
                nc.sync.dma_start(qte[:, lo:hi], qte_d[h, :, lo:hi])
            vex = head_pool.tile([P, NKT * E], F32R, tag="vex", name=f"vex{h}")
            nc.sync.dma_start(vex[:], vex_d[h])
            tiles[h] = (qte, kte, vex)

        load_head(0)
        steps = []
        for h in range(HPC):
            for qc in range(NQC):
                qs = qc * QC
                if h == HPC - 1 and qc == NQC - 1:
                    steps.append((h, qs, QC // 2))
                    steps.append((h, qs + QC // 2, QC // 2))
                else:
                    steps.append((h, qs, QC))

        pending = None  # (h, qs, w, exps, expst) awaiting B2 + the C phases
        for h, qs, w in steps:
            qte, kte, vex = tiles[h]
            exps = phase_A(qte, kte, qs, w)
            if pending is not None:
                ph, pqs, pw, pexps, pexpst = pending
                phase_B2(pexps, pexpst, pw)
                precip = phase_C1(tiles[ph][2], ph, pqs, pexps, pexpst, pw)
            expst = phase_B1(qte, kte, qs, w)
            if qs == 0 and h + 1 < HPC:
                # emit next head's loads early; the pool slot WAR gates
                # the actual DMA start, so this just un-delays it.
                load_head(h + 1)
            if pending is not None:
                for t in range(pw // P):
                    phase_C2t(ph, pqs, pexps, precip, t)
            pending = (h, qs, w, exps, expst)

        ph, pqs, pw, pexps, pexpst = pending
        phase_B2(pexps, pexpst, pw)
        precip = phase_C1(tiles[ph][2], ph, pqs, pexps, pexpst, pw)
        # final flush: normalize + DMA in half-tiles so the tail DMAs
        # overlap the remaining normalizes
        for t in range(pw // P):
            qt0 = pqs + t * P
            for hf in range(2):
                sl = slice(hf * S // 2, (hf + 1) * S // 2)
                nc.vector.tensor_scalar_mul(
                    pexps[t][:, sl], pexps[t][:, sl], precip[:, t:t + 1]
                )
                nc.sync.dma_start(
                    probs_d[ph, qt0:qt0 + P, sl], pexps[t][:, sl]
                )

    nc.compile()
    return nc


def _get_nc():
    if "nc" not in _CACHE:
        _CACHE["nc"] = _build_bass()
    return _CACHE["nc"]


def kernel(query, key, value, mask, trace=False):
    query = np.asarray(query, dtype=np.float32).reshape(B * H, S, D)
    key_ = np.asarray(key, dtype=np.float32).reshape(B * H, S, D)
    value = np.asarray(value, dtype=np.float32).reshape(B * H, S, D)
    mask_f = np.asarray(mask, dtype=np.float32).reshape(B, S)

    in_maps = []
    for c in range(N_CORES):
        hh = slice(c * HPC, (c + 1) * HPC)
        qte = np.ones((HPC, E, S), dtype=np.float32)
        qte[:, :D, :] = query[hh].transpose(0, 2, 1)
        kte = np.empty((HPC, E, S), dtype=np.float32)
        kte[:, :D, :] = key_[hh].transpose(0, 2, 1)
        for i, hf in enumerate(range(c * HPC, (c + 1) * HPC)):
            b = hf // H
            kte[i, D, :] = (mask_f[b] - 1.0) * (-MASKVAL)
        vex = np.ones((HPC, P, NKT, E), dtype=np.float32)
        vex[:, :, :, :D] = value[hh].reshape(HPC, NKT, P, D).transpose(0, 2, 1, 3)
        in_maps.append(
            {
                "qte": np.ascontiguousarray(qte),
                "kte": np.ascontiguousarray(kte),
                "vex": np.ascontiguousarray(vex.reshape(HPC, P, NKT * E)),
            }
        )

    nc = _get_nc()
    res = run_bass_kernel_spmd(nc, in_maps, list(range(N_CORES)), trace=trace)
    results = res.results

    out = np.empty((B * H, S, D), dtype=np.float32)
    probs = np.empty((B * H, S, S), dtype=np.float32)
    for c in range(N_CORES):
        out[c * HPC:(c + 1) * HPC] = results[c]["out"]
        probs[c * HPC:(c + 1) * HPC] = results[c]["probs"]

    out = out.reshape(B, H, S, D)
    probs = probs.reshape(B, H, S, S)
    if trace:
        return (out, probs), res
    return (out, probs)


# revision 53
# speedup vs baseline: 1.0018x; 1.0018x over previous
"""Attention kernel (out, probs) for Trainium2, 8-core SPMD.

Problem: B=2, H=16, S=2048, D=64 fp32 attention with a 0/1 key mask,
returning BOTH the attention output [B,H,S,D] and the full softmax
probabilities [B,H,S,S].

Sharding: the 32 (b,h) pairs are split 4-per-core (head parallel, no
cross-core communication).

Per-core algorithm, per head (S=2048 split into 4 q-chunks of 512):
  1. S pass: scores [128q, 2048k] via matmul(lhsT=QTe, rhs=KTe) where
     QTe=[Q^T; ones] and KTe=[K^T; maskterm] carry an extra contraction
     row adding maskterm[k] = (mask[k]-1)*8e9 to every score (masked
     keys underflow to exactly 0 after exp).  ScalarE exp(0.125*x) ->
     unnormalized expS tiles (float32).
  2. Transposed copies expST[k,q] needed for the P.V contraction come
     from two sources (balancing ScalarE vs PE/VectorE):
       - K_ACT tiles: recompute scores^T via matmul(lhsT=KTe, rhs=QTe)
         + a second ScalarE exp (writes float32r directly),
       - K_TR tiles: PE-transpose 128x128 blocks of expS into PSUM,
         VectorE-copy to SBUF as float32r.
  3. PV: outT[65, 512q] = sum_k [V | 1]^T @ expST -- the ones column
     yields the softmax denominators for free.
  4. PE-transpose outT chunks -> [128q, 65]; VectorE reciprocal of the
     denominator column; out rows scaled by recip -> DMA.
  5. probs = expS * recip[q] in-place on VectorE -> contiguous 1 MiB
     DMA per 128-row q-tile.

All matmuls run in float32r (TF32-like, 1 cycle/row); resulting rel
error vs the fp32 reference is ~2e-4, far inside the 2e-2 gate.
"""

import sys

try:
    import concourse.bass  # noqa: F401
except ImportError:
    sys.path.insert(0, "/opt/trn_rl_repo")

from contextlib import ExitStack

import numpy as np

import concourse.mybir as mybir
import concourse.tile as tile
from concourse import bacc
from concourse.bass_utils import run_bass_kernel_spmd
from concourse.masks import make_identity

B, H, S, D = 2, 16, 2048, 64
N_CORES = 8
HPC = (B * H) // N_CORES  # heads per core = 4
P = 128
QC = 512                  # q-chunk size
NQC = S // QC             # 4 q-chunks per head
NKT = S // P              # 16 k-tiles of 128
E = D + 1                 # 65: V columns + ones column
F32 = mybir.dt.float32
F32R = mybir.dt.float32r
EXP = mybir.ActivationFunctionType.Exp
SCALE = 0.125             # 1/sqrt(D)
MASKVAL = -8.0e9          # * SCALE = -1e9, matching the reference NEG
K_TR = 7                  # kt tiles per q-chunk via transpose path (rest via 2nd exp)

_CACHE = {}


def _build_bass(k_tr=K_TR):
    k_act = NKT - k_tr    # kt tiles via matmul+exp path
    nc = bacc.Bacc(None)
    qte_d = nc.dram_tensor("qte", [HPC, E, S], F32R, kind="ExternalInput")
    kte_d = nc.dram_tensor("kte", [HPC, E, S], F32R, kind="ExternalInput")
    vex_d = nc.dram_tensor("vex", [HPC, P, NKT * E], F32R, kind="ExternalInput")
    probs_d = nc.dram_tensor("probs", [HPC, S, S], F32, kind="ExternalOutput")
    out_d = nc.dram_tensor("out", [HPC, S, D], F32, kind="ExternalOutput")

    with tile.TileContext(nc) as tc, ExitStack() as ctx:
        const_pool = ctx.enter_context(tc.tile_pool(name="const", bufs=1))
        head_pool = ctx.enter_context(tc.tile_pool(name="head", bufs=3))
        # PSUM budget (8 banks):
        #   scores [128,1024] x2 bufs                      = 4 banks
        #   trdst  [128, 512] x2 bufs                      = 2 banks
        #   outt   [128, 512] x2 (shared with otr fixups)  = 2 banks
        scores_ps = ctx.enter_context(tc.tile_pool(name="scores", bufs=2, space="PSUM"))
        trdst_ps = ctx.enter_context(tc.tile_pool(name="trdst", bufs=2, space="PSUM"))
        outt_ps = ctx.enter_context(tc.tile_pool(name="outt", bufs=2, space="PSUM"))
        exps_pool = ctx.enter_context(tc.tile_pool(name="exps", bufs=10))
        expst_pool = ctx.enter_context(tc.tile_pool(name="expst", bufs=2 * NKT + 2))
        small_pool = ctx.enter_context(tc.tile_pool(name="small", bufs=2))

        ident = const_pool.tile([P, P], F32)
        make_identity(nc, ident)
        # warm the ACT exp table during the input-load window so the first
        # real exp doesn't pay the ACT_TABLE_LOAD inline
        warm = const_pool.tile([1, 1], F32)
        nc.vector.memset(warm[:], 0.0)
        nc.scalar.activation(warm[:], warm[:], EXP)

        def phase_A(qte, kte, qs, w):
            # S pass: scores [128q, 1024k] tiles + exp (unnormalized)
            exps = []  # w//P tiles of [128q, 2048k] fp32, one per q-tile
            for t in range(w // P):
                qt0 = qs + t * P
                es = exps_pool.tile([P, S], F32, tag="exps")
                for half in range(2):
                    sps = scores_ps.tile([P, 2 * QC], F32, tag="sc")
                    for j in range(2):
                        ks = (half * 2 + j) * QC
                        nc.tensor.matmul(
                            sps[:, j * QC:(j + 1) * QC],
                            qte[:, qt0:qt0 + P],
                            kte[:, ks:ks + QC],
                            start=True,
                            stop=True,
                        )
                    nc.scalar.activation(
                        es[:, half * 2 * QC:(half + 1) * 2 * QC],
                        sps[:],
                        EXP,
                        scale=SCALE,
                    )
                exps.append(es)
            return exps

        def phase_B1(qte, kte, qs, w, n_act=None):
            # expST tiles [128k, w], matmul + 2nd exp path for kt < n_act
            n_act = k_act if n_act is None else n_act
            expst = [None] * NKT
            for kp in range((n_act + 1) // 2):
                st = scores_ps.tile([P, 2 * QC], F32, tag="sc")
                n_in_pair = min(2, n_act - 2 * kp)
                for j in range(n_in_pair):
                    kt = 2 * kp + j
                    nc.tensor.matmul(
                        st[:, j * w:(j + 1) * w],
                        kte[:, kt * P:(kt + 1) * P],
                        qte[:, qs:qs + w],
                        start=True,
                        stop=True,
                    )
                est = expst_pool.tile(
                    [P, 2 * QC], F32R, tag="est", bufs=max((k_act + 1) // 2 + 3, NKT // 2 + 1)
                )
                nc.scalar.activation(
                    est[:, :n_in_pair * w], st[:, :n_in_pair * w], EXP, scale=SCALE
                )
                for j in range(n_in_pair):
                    expst[2 * kp + j] = est[:, j * w:(j + 1) * w]
            return expst

        def phase_B2(exps, expst, w):
            # PE-transpose path for the kt tiles B1 didn't fill -- emitted one
            # pipeline step AFTER phase_A so the transposes never wait on
            # same-step ACT
            for kt in range(NKT):
                if expst[kt] is not None:
                    continue
                td = trdst_ps.tile([P, QC], F32, tag="trdst")
                for t in range(w // P):
                    nc.tensor.transpose(
                        td[:, t * P:(t + 1) * P],
                        exps[t][:, kt * P:(kt + 1) * P],
                        ident[:],
                    )
                etr = expst_pool.tile([P, QC], F32R, tag="etr", bufs=k_tr + 2)
                nc.vector.tensor_copy(etr[:, :w], td[:, :w])
                expst[kt] = etr[:, :w]

        def phase_C1(vex, h, qs, exps, expst, w):
            # PV: outT[65, w] = sum_k [V|1]^T expST; the same PSUM bank is
            # reused afterwards for the [128, 65] out-transposes.
            fix = outt_ps.tile([P, QC], F32, tag="outt")
            outt = fix[:E, :w]
            for kt in range(NKT):
                nc.tensor.matmul(
                    outt,
                    vex[:, kt * E:(kt + 1) * E],
                    expst[kt],
                    start=(kt == 0),
                    stop=(kt == NKT - 1),
                )
            outt_sb = small_pool.tile([E, QC], F32, tag="outt_sb")
            nc.vector.tensor_copy(outt_sb[:, :w], outt)

            # transpose fixup + denominators + out (batched small ops + DMA)
            nt = w // P
            recip = small_pool.tile([P, NQC], F32, tag="recip")
            outsb = small_pool.tile([P, NQC * D], F32, tag="outsb")
            for t in range(nt):
                nc.tensor.transpose(
                    fix[:, t * E:(t + 1) * E],
                    outt_sb[:, t * P:(t + 1) * P],
                    ident[:E, :E],
                )
            fix3 = fix[:, :nt * E].rearrange("p (t e) -> p t e", e=E)
            nc.vector.reciprocal(recip[:, :nt], fix3[:, :, D])
            nc.vector.tensor_mul(
                outsb[:, :nt * D].rearrange("p (t d) -> p t d", d=D),
                fix3[:, :, :D],
                recip[:, :nt, None].to_broadcast((P, nt, D)),
            )
            nc.sync.dma_start(
                out_d[h, qs:qs + w, :].rearrange("(t p) d -> p t d", p=P),
                outsb[:, :nt * D].rearrange("p (t d) -> p t d", d=D),
            )
            return recip

        def phase_C2t(h, qs, exps, recip, t):
            # probs: normalize expS in place, DMA out (one q-tile)
            qt0 = qs + t * P
            nc.vector.tensor_scalar_mul(
                exps[t][:], exps[t][:], recip[:, t:t + 1]
            )
            nc.sync.dma_start(probs_d[h, qt0:qt0 + P, :], exps[t][:])

        # Software-pipelined emission over (head, q-offset, width) steps: the
        # final q-chunk is split into two 256-wide steps to shorten the
        # pipeline drain (PV/normalize/DMA of the very last unit).
        tiles = {}

        def load_head(h):
            qte = head_pool.tile([E, S], F32R, tag="qte", name=f"qte{h}")
            kte = head_pool.tile([E, S], F32R, tag="kte", name=f"kte{h}")
            for lo, hi in ((0, S // 2), (S // 2, S)):
                nc.sync.dma_start(kte[:, lo:hi], kte_d[h, :, lo:hi])
                nc.sync.dma_start(qte[:, lo:hi], qte_d[h, :, lo:hi])
            vex = head_pool.tile([P, NKT * E], F32R, tag="vex", name=f"vex{h}")
            nc.sync.dma_start(vex[:], vex_d[h])
            tiles[h] = (qte, kte, vex)

        load_head(0)
        steps = []
        for h in range(HPC):
            for qc in range(NQC):
                qs = qc * QC
                if h == HPC - 1 and qc == NQC - 1:
                    steps.append((h, qs, QC // 2, NKT))
                    steps.append((h, qs + QC // 2, QC // 2, NKT))
                else:
                    steps.append((h, qs, QC, None))

        pending = None  # (h, qs, w, exps, expst) awaiting B2 + the C phases
        for h, qs, w, na in steps:
            qte, kte, vex = tiles[h]
            exps = phase_A(qte, kte, qs, w)
            if pending is not None:
                ph, pqs, pw, pexps, pexpst = pending
                phase_B2(pexps, pexpst, pw)
                precip = phase_C1(tiles[ph][2], ph, pqs, pexps, pexpst, pw)
            expst = phase_B1(qte, kte, qs, w, n_act=na)
            if qs == 0 and h + 1 < HPC:
                # emit next head's loads early; the pool slot WAR gates
                # the actual DMA start, so this just un-delays it.
                load_head(h + 1)
            if pending is not None:
                for t in range(pw // P):
                    phase_C2t(ph, pqs, pexps, precip, t)
            pending = (h, qs, w, exps, expst)

        ph, pqs, pw, pexps, pexpst = pending
        phase_B2(pexps, pexpst, pw)
        precip = phase_C1(tiles[ph][2], ph, pqs, pexps, pexpst, pw)
        # final flush: normalize + DMA in half-tiles so the tail DMAs
        # overlap the remaining normalizes
        for t in range(pw // P):
            qt0 = pqs + t * P
            for hf in range(2):
                sl = slice(hf * S // 2, (hf + 1) * S // 2)
                nc.vector.tensor_scalar_mul(
                    pexps[t][:, sl], pexps[t][:, sl], precip[:, t:t + 1]
                )
                nc.sync.dma_start(
                    probs_d[ph, qt0:qt0 + P, sl], pexps[t][:, sl]
                )

    nc.compile()
    return nc


def _get_nc():
    if "nc" not in _CACHE:
        _CACHE["nc"] = _build_bass()
    return _CACHE["nc"]


def kernel(query, key, value, mask, trace=False):
    query = np.asarray(query, dtype=np.float32).reshape(B * H, S, D)
    key_ = np.asarray(key, dtype=np.float32).reshape(B * H, S, D)
    value = np.asarray(value, dtype=np.float32).reshape(B * H, S, D)
    mask_f = np.asarray(mask, dtype=np.float32).reshape(B, S)

    in_maps = []
    for c in range(N_CORES):
        hh = slice(c * HPC, (c + 1) * HPC)
        qte = np.ones((HPC, E, S), dtype=np.float32)
        qte[:, :D, :] = query[hh].transpose(0, 2, 1)
        kte = np.empty((HPC, E, S), dtype=np.float32)
        kte[:, :D, :] = key_[hh].transpose(0, 2, 1)
        for i, hf in enumerate(range(c * HPC, (c + 1) * HPC)):
            b = hf // H
            kte[i, D, :] = (mask_f[b] - 1.0) * (-MASKVAL)
        vex = np.ones((HPC, P, NKT, E), dtype=np.float32)
        vex[:, :, :, :D] = value[hh].reshape(HPC, NKT, P, D).transpose(0, 2, 1, 3)
        in_maps.append(
            {
                "qte": np.ascontiguousarray(qte),
                "kte": np.ascontiguousarray(kte),
                "vex": np.ascontiguousarray(vex.reshape(HPC, P, NKT * E)),
            }
        )

    nc = _get_nc()
    res = run_bass_kernel_spmd(nc, in_maps, list(range(N_CORES)), trace=trace)
    results = res.results

    out = np.empty((B * H, S, D), dtype=np.float32)
    probs = np.empty((B * H, S, S), dtype=np.float32)
    for c in range(N_CORES):
        out[c * HPC:(c + 1) * HPC] = results[c]["out"]
        probs[c * HPC:(c + 1) * HPC] = results[c]["probs"]

    out = out.reshape(B, H, S, D)
    probs = probs.reshape(B, H, S, S)
    if trace:
        return (out, probs), res
    return (out, probs)
